# revision 1
# baseline (speedup 1.0000x reference)
"""Trainium2 Bass kernel for multi-head attention (B=4, N=2048, H=1024, 16 heads).

Sharding: 8 cores = 4 batches x 2 head-groups (8 heads each).  Each core:
  - computes q/k/v projections for its 8 heads from its batch's x,
  - applies RoPE, runs SDPA (scores kept transposed: [kv, q]-layout so the
    softmax denominator comes from an extra ones-column in the PV matmul),
  - computes the partial output projection over its 512 head-dim columns.
Host sums the two partials per batch.

All matmuls run in bf16 with fp32 PSUM accumulation.
"""

import numpy as np
import ml_dtypes

B, N, H, HEADS, D = 4, 2048, 1024, 16, 64
N_CORES = 8
HPC = 8          # heads per core
PAIRS = HPC // 2

_BF16 = ml_dtypes.bfloat16

_NC_CACHE = {}

# adjacent-pair swap for rotate-half on interleaved head dims
_PAIR_SWAP_MASK = [i ^ 1 for i in range(32)]

# head-dim interleave [d0, d32, d1, d33, ...] so rotate-half partners are
# adjacent partitions
_PERM = np.empty(64, dtype=np.int64)
_PERM[0::2] = np.arange(32)
_PERM[1::2] = 32 + np.arange(32)
# sign of the sin term per interleaved row: even rows take -sin (partner is
# the upper half), odd rows take +sin
_SIGN = np.where(np.arange(64) % 2 == 0, -1.0, 1.0).astype(np.float32)


def build_bass(n_tokens=N):
    """Build the per-core Bass module (same program on all 8 cores)."""
    from contextlib import ExitStack
    import concourse.bass as bass
    import concourse.mybir as mybir
    import concourse.tile as tile
    from concourse import bacc

    f32 = mybir.dt.float32
    bf16 = mybir.dt.bfloat16
    Exp = mybir.ActivationFunctionType.Exp

    NT = n_tokens
    TB = NT // 128        # 128-token blocks
    TC = NT // 512        # 512-token chunks
    KV = TB
    QC = TC

    nc = bacc.Bacc()

    x_bf = nc.dram_tensor("x_bf", [128, 8, NT], bf16, kind="ExternalInput")
    wqkv = nc.dram_tensor("wqkv", [128, 8, 3 * 512], bf16, kind="ExternalInput")
    wo = nc.dram_tensor("wo", [128, 4, H], bf16, kind="ExternalInput")
    cos_r = nc.dram_tensor("cos_r", [128, NT], f32, kind="ExternalInput")
    sin_r = nc.dram_tensor("sin_r", [128, NT], f32, kind="ExternalInput")
    y = nc.dram_tensor("y", [NT, H], f32, kind="ExternalOutput")

    with ExitStack() as ctx:
        tc_ = ctx.enter_context(tile.TileContext(nc))

        sing = ctx.enter_context(tc_.tile_pool(name="resident", bufs=1))
        qkv_ps = ctx.enter_context(tc_.tile_pool(name="qkv_ps", bufs=2, space="PSUM"))
        sc_ps = ctx.enter_context(tc_.tile_pool(name="sc_ps", bufs=2, space="PSUM"))
        pv_ps = ctx.enter_context(tc_.tile_pool(name="pv_ps", bufs=2, space="PSUM"))
        rope_sb = ctx.enter_context(tc_.tile_pool(name="rope_sb", bufs=2))
        p_pool = ctx.enter_context(tc_.tile_pool(name="p_pool", bufs=6))
        norm_sb = ctx.enter_context(tc_.tile_pool(name="norm_sb", bufs=2))
        y_pool = ctx.enter_context(tc_.tile_pool(name="y_pool", bufs=2))
        bc_dram = ctx.enter_context(tc_.tile_pool(name="bc_dram", bufs=2, space="DRAM"))

        # -------- resident SBUF tensors --------
        xT = sing.tile([128, 8, NT], bf16, name="xT", tag="xT")
        wqkv_sb = sing.tile([128, 8, 3 * 512], bf16, name="wqkv_sb", tag="wqkv_sb")
        wo_sb = sing.tile([128, 4, H], bf16, name="wo_sb", tag="wo_sb")
        cos_sb = sing.tile([128, NT], f32, name="cos_sb", tag="cos_sb")
        sin_sb = sing.tile([128, NT], f32, name="sin_sb", tag="sin_sb")
        qT = [sing.tile([128, NT], bf16, name=f"qT{p}", tag=f"qT{p}") for p in range(PAIRS)]
        kT = [sing.tile([128, NT], bf16, name=f"kT{p}", tag=f"kT{p}") for p in range(PAIRS)]
        # v_aug[p][part, kvblock, head, 65]: col 0 = ones (softmax denominator),
        # cols 1..64 = v head dims
        vaug = [sing.tile([128, KV, 2, 65], bf16, name=f"vaug{p}", tag=f"vaug{p}")
                for p in range(PAIRS)]
        attn = [sing.tile([128, NT], bf16, name=f"attn{p}", tag=f"attn{p}")
                for p in range(PAIRS)]

        def qk_group(p, off, dst, tcn):
            """q or k projection for one head pair and one 512-token chunk,
            followed by RoPE into dst."""
            tsl = slice(tcn * 512, (tcn + 1) * 512)
            ps = qkv_ps.tile([128, 512], f32, name="ps_qk", tag="qkvps")
            for kc in range(8):
                nc.tensor.matmul(
                    ps,
                    lhsT=wqkv_sb[:, kc, off + p * 128: off + (p + 1) * 128],
                    rhs=xT[:, kc, tsl],
                    start=(kc == 0), stop=(kc == 7),
                )
            # RoPE: out = ps * cos + rot(ps) * sin_signed.
            # Head dims are host-interleaved [d0,d32,d1,d33,...], so
            # rotate-half is an adjacent-pair swap: a single DVE
            # stream_shuffle (within-quadrant permutation).
            rot = rope_sb.tile([128, 512], f32, name="rot", tag="rot")
            nc.vector.stream_shuffle(rot, ps, mask=_PAIR_SWAP_MASK)
            t1 = rope_sb.tile([128, 512], f32, name="t1", tag="t1")
            nc.vector.tensor_mul(t1, ps, cos_sb[:, tsl])
            t2 = rope_sb.tile([128, 512], f32, name="t2", tag="t2")
            nc.gpsimd.tensor_mul(t2, rot, sin_sb[:, tsl])
            nc.gpsimd.tensor_add(dst[:, tsl], t1, t2)

        # preload the exp table set while the input DMAs run (first use of a
        # table set costs ~2.7us on ScalarE)
        warm = norm_sb.tile([1, 1], f32, name="warm", tag="warm", bufs=1)
        nc.vector.memset(warm, 0.0)
        nc.scalar.activation(warm, warm, Exp)

        # -------- input loads + v projection + pair-0 q/k, interleaved
        # per 512-token chunk so PE work starts as soon as the first x
        # chunk is transposed.  Two HWDGE rings: weights on scalar, x on sync.
        nc.scalar.dma_start(out=wqkv_sb[:, 0:4, 1024:1536], in_=wqkv[:, 0:4, 1024:1536])
        nc.scalar.dma_start(out=wqkv_sb[:, 4:8, 1024:1536], in_=wqkv[:, 4:8, 1024:1536])
        for p in range(PAIRS):
            nc.vector.memset(vaug[p][:, :, :, 0:1], 1.0)

        # x is host-transposed: plain contiguous loads, one per 512-token
        # chunk, on the sync ring; weight copies on the scalar ring
        for s in (slice(0, 512), slice(512, 1024)):
            nc.scalar.dma_start(out=wqkv_sb[:, :, s], in_=wqkv[:, :, s])
        nc.scalar.dma_start(out=cos_sb, in_=cos_r[:, :])
        nc.scalar.dma_start(out=sin_sb, in_=sin_r[:, :])
        # first chunk split in half so the first matmuls start sooner
        nc.sync.dma_start(out=xT[:, :, 0:256], in_=x_bf[:, :, 0:256])
        nc.sync.dma_start(out=xT[:, :, 256:512], in_=x_bf[:, :, 256:512])
        for tcn in range(1, TC):
            tsl = slice(tcn * 512, (tcn + 1) * 512)
            nc.sync.dma_start(out=xT[:, :, tsl], in_=x_bf[:, :, tsl])

        # -------- v projection (natural layout), all 8 heads at once --------
        for tb in range(TB):
            ps_v = qkv_ps.tile([128, 512], f32, name="ps_v", tag="qkvps")
            for kc in range(8):
                nc.tensor.matmul(
                    ps_v,
                    lhsT=xT[:, kc, tb * 128:(tb + 1) * 128],
                    rhs=wqkv_sb[:, kc, 1024:1536],
                    start=(kc == 0), stop=(kc == 7),
                )
            ps_v4 = ps_v.rearrange("a (p h d) -> a p h d", p=PAIRS, h=2)
            for p in range(PAIRS):
                nc.vector.tensor_copy(out=vaug[p][:, tb, :, 1:65],
                                      in_=ps_v4[:, p, :, :])

        # q/k projections + RoPE for pair 0; later pairs' projections are
        # emitted interleaved into the previous pair's SDPA loop below
        for off, dst in ((0, qT[0]), (512, kT[0])):
            for tcn in range(TC):
                qk_group(0, off, dst, tcn)

        # out-projection weights are only needed at the end; load them after
        # the startup-critical transfers
        nc.scalar.dma_start(out=wo_sb, in_=wo[:, :, :])

        for p in range(PAIRS):
            # -------- SDPA for this head pair --------
            for qc in range(QC):
                qsl = slice(qc * 512, (qc + 1) * 512)
                o_ps = [pv_ps.tile([65, 512], f32, name=f"o_ps{hh}", tag="pv")
                        for hh in range(2)]
                # software-pipelined: PV lags one kv step so the PE stream
                # issues the next scores before the exp-gated PV matmuls
                pt_prev = None
                for kv in range(KV):
                    ksl = slice(kv * 128, (kv + 1) * 128)
                    sc = sc_ps.tile([128, 2, 512], f32, name="sc", tag="sc")
                    nc.tensor.matmul(sc[:, 0, :], lhsT=kT[p][0:64, ksl],
                                     rhs=qT[p][0:64, qsl])
                    nc.tensor.matmul(sc[:, 1, :], lhsT=kT[p][64:128, ksl],
                                     rhs=qT[p][64:128, qsl])
                    pt = p_pool.tile([128, 2, 512], bf16, name="pt", tag="pt")
                    nc.scalar.activation(pt, sc, Exp)
                    if pt_prev is not None:
                        for hh in range(2):
                            nc.tensor.matmul(
                                o_ps[hh],
                                lhsT=vaug[p][:, kv - 1, hh, :],
                                rhs=pt_prev[:, hh, :],
                                start=(kv == 1), stop=False,
                            )
                    pt_prev = pt
                for hh in range(2):
                    nc.tensor.matmul(
                        o_ps[hh],
                        lhsT=vaug[p][:, KV - 1, hh, :],
                        rhs=pt_prev[:, hh, :],
                        start=False, stop=True,
                    )
                # normalize: row 0 of o_ps = sum(exp); rows 1..64 = unnormalized out.
                # Stage to SBUF first so the PSUM accumulators free up
                # immediately; the broadcast round trip happens off that path.
                stgU = [norm_sb.tile([65, 512], f32, name=f"stgU{hh}", tag=f"stgU{hh}")
                        for hh in range(2)]
                rc = norm_sb.tile([1, 2, 512], f32, name="rc", tag="rc")
                for hh in range(2):
                    nc.vector.tensor_copy(out=stgU[hh], in_=o_ps[hh][0:65, :])
                    nc.vector.reciprocal(rc[:, hh, :], stgU[hh][0:1, :])
                # broadcast rc across partitions via a DRAM round trip
                # (partition-step-0 source APs are only legal from DRAM)
                rcd = bc_dram.tile([1, 1024], f32, name="rcd", tag="rcd")
                nc.sync.dma_start(out=rcd, in_=rc)
                bc = norm_sb.tile([65, 2, 512], f32, name="bc", tag="bc")
                nc.sync.dma_start(out=bc, in_=rcd.to_broadcast((65, 1024)))
                for hh in range(2):
                    stg = norm_sb.tile([65, 512], bf16, name="stg", tag="stg")
                    # row 0 multiplies the sum by its reciprocal (discarded);
                    # base partition must be 0 for the DVE op.
                    nc.vector.tensor_mul(stg[0:65, :], stgU[hh][0:65, :], bc[:, hh, :])
                    nc.sync.dma_start(out=attn[p][hh * 64:(hh + 1) * 64, qsl],
                                      in_=stg[1:65, :])
                # interleave the next pair's q/k projection work so the PE
                # can fill ACT-paced gaps with it
                if p + 1 < PAIRS:
                    qk_group(p + 1, 0, qT[p + 1], qc)
                    qk_group(p + 1, 512, kT[p + 1], qc)

        # -------- output projection (partial over this core's 512 columns) --------
        for tb in range(TB):
            tsl = slice(tb * 128, (tb + 1) * 128)
            ysb = y_pool.tile([128, 1024], f32, name="ysb", tag="ysb", bufs=3)
            for oc in range(2):
                osl = slice(oc * 512, (oc + 1) * 512)
                ps_y = qkv_ps.tile([128, 512], f32, name="ps_y", tag="qkvps")
                for p in range(PAIRS):
                    nc.tensor.matmul(
                        ps_y,
                        lhsT=attn[p][:, tsl],
                        rhs=wo_sb[:, p, osl],
                        start=(p == 0), stop=(p == PAIRS - 1),
                    )
                nc.vector.tensor_copy(out=ysb[:, osl], in_=ps_y)
            nc.sync.dma_start(out=y[tsl, :], in_=ysb)

    nc.finalize()
    return nc


def get_bass(n_tokens=N):
    if n_tokens not in _NC_CACHE:
        _NC_CACHE[n_tokens] = build_bass(n_tokens)
    return _NC_CACHE[n_tokens]


def host_prep(x, rotary_emb, w_qkv, w_out, n_tokens=N, n_batches=B):
    """Build the 8 per-core input maps from the full-size inputs."""
    x = np.asarray(x, dtype=np.float32)
    rotary_emb = np.asarray(rotary_emb, dtype=np.float32)
    w_qkv = np.asarray(w_qkv, dtype=np.float32)
    w_out = np.asarray(w_out, dtype=np.float32)

    x_bf = x.astype(_BF16)
    # pre-transposed per batch: [p, hc, t] = x[b, t, hc*128+p]
    x_t_dev = [np.ascontiguousarray(
        x_bf[b].T.reshape(8, 128, n_tokens).transpose(1, 0, 2))
        for b in range(n_batches)]

    # rope tables in the interleaved head-dim order (duplicated for the two
    # heads sharing a partition block)
    cos_t = np.cos(rotary_emb).T.astype(np.float32)[_PERM]     # [64, NT]
    sin_t = (np.sin(rotary_emb).T.astype(np.float32)[_PERM]) * _SIGN[:, None]
    cos128 = np.ascontiguousarray(np.concatenate([cos_t, cos_t], axis=0))
    sin128 = np.ascontiguousarray(np.concatenate([sin_t, sin_t], axis=0))

    # per-head column permutation of the q/k projection outputs
    qk_col_perm = (np.arange(8)[:, None] * 64 + _PERM[None, :]).reshape(-1)

    per_group = []
    for g in range(2):
        gs = slice(g * 512, (g + 1) * 512)
        wq_t = w_qkv[0 * H:1 * H][gs].T * (1.0 / np.sqrt(D))   # [1024, 512], scale folded
        wk_t = w_qkv[1 * H:2 * H][gs].T
        wv_t = w_qkv[2 * H:3 * H][gs].T
        wq_t = wq_t[:, qk_col_perm]
        wk_t = wk_t[:, qk_col_perm]
        wqkv_t = np.concatenate([wq_t, wk_t, wv_t], axis=1)    # [1024, 1536]
        wqkv_dev = np.ascontiguousarray(
            wqkv_t.reshape(8, 128, 3 * 512).transpose(1, 0, 2)).astype(_BF16)
        wo_t = w_out[:, gs].T                                   # [512, 1024]
        wo_dev = np.ascontiguousarray(
            wo_t.reshape(4, 128, H).transpose(1, 0, 2)).astype(_BF16)
        per_group.append((wqkv_dev, wo_dev))

    in_maps = []
    for c in range(2 * n_batches):
        b, g = c // 2, c % 2
        wqkv_dev, wo_dev = per_group[g]
        in_maps.append({
            "x_bf": x_t_dev[b],
            "wqkv": wqkv_dev,
            "wo": wo_dev,
            "cos_r": cos128,
            "sin_r": sin128,
        })
    return in_maps


def run_on_hw(in_maps, n_tokens=N, trace=False):
    from concourse.bass_utils import run_bass_kernel_spmd
    nc = get_bass(n_tokens)
    core_ids = list(range(len(in_maps)))
    try:
        return run_bass_kernel_spmd(nc, in_maps, core_ids, trace=trace)
    except ModuleNotFoundError:
        # axon NTFF profiling hook unavailable in this container
        return run_bass_kernel_spmd(nc, in_maps, core_ids, trace=False)


def make_sharded_callable(in_maps, n_tokens=N, donate=True):
    """Replicates bass2jax.run_bass_via_pjrt's multi-core path but returns a
    reusable jitted callable + prepared host args, for steady-state timing."""
    import jax
    import numpy as _np
    from jax.sharding import Mesh, PartitionSpec, NamedSharding
    from jax.experimental.shard_map import shard_map
    import concourse.mybir as mybir
    from concourse import bass2jax

    bass2jax.install_neuronx_cc_hook()
    nc = get_bass(n_tokens)
    n_cores = len(in_maps)

    partition_name = nc.partition_id_tensor.name if nc.partition_id_tensor else None
    in_names, out_names, out_avals, zero_outs = [], [], [], []
    for alloc in nc.m.functions[0].allocations:
        if not isinstance(alloc, mybir.MemoryLocationSet):
            continue
        name = alloc.memorylocations[0].name
        if alloc.kind == "ExternalInput":
            if name != partition_name:
                in_names.append(name)
        elif alloc.kind == "ExternalOutput":
            shape = tuple(alloc.tensor_shape)
            dtype = mybir.dt.np(alloc.dtype)
            out_names.append(name)
            out_avals.append(jax.core.ShapedArray(shape, dtype))
            zero_outs.append(_np.zeros(shape, dtype))
    n_params = len(in_names)
    n_outs = len(out_avals)
    all_in_names = list(in_names) + out_names
    if partition_name is not None:
        all_in_names.append(partition_name)

    def _body(*args):
        operands = list(args)
        if partition_name is not None:
            operands.append(bass2jax.partition_id_tensor())
        outs = bass2jax._bass_exec_p.bind(
            *operands,
            out_avals=tuple(out_avals),
            in_names=tuple(all_in_names),
            out_names=tuple(out_names),
            lowering_input_output_aliases=(),
            sim_require_finite=True,
            sim_require_nnan=True,
            nc=nc,
        )
        return tuple(outs)

    devices = jax.devices()[:n_cores]
    mesh = Mesh(_np.asarray(devices), ("core",))
    in_specs = (PartitionSpec("core"),) * (n_params + n_outs)
    out_specs = (PartitionSpec("core"),) * n_outs
    donate_idx = tuple(range(n_params, n_params + n_outs)) if donate else ()
    sharded = jax.jit(
        shard_map(_body, mesh=mesh, in_specs=in_specs, out_specs=out_specs,
                  check_rep=False),
        donate_argnums=donate_idx,
        keep_unused=True,
    )
    per_core = [[_np.asarray(m[name]) for name in in_names] for m in in_maps]
    concat_in = [
        _np.concatenate([per_core[c][i] for c in range(n_cores)], axis=0)
        for i in range(n_params)
    ]
    concat_zeros = [
        _np.zeros((n_cores * z.shape[0], *z.shape[1:]), z.dtype) for z in zero_outs
    ]
    sharding = NamedSharding(mesh, PartitionSpec("core"))
    return sharded, concat_in, concat_zeros, sharding, out_names, out_avals


def time_kernel(in_maps, n_tokens=N, iters=6):
    """Steady-state wall time of one sharded NEFF execution (device-resident
    inputs; measures dispatch + exec + sync)."""
    import time as _time
    import jax
    sharded, concat_in, concat_zeros, sharding, _, _ = make_sharded_callable(
        in_maps, n_tokens)
    times = []
    for _ in range(iters):
        args = [jax.device_put(a, sharding) for a in concat_in + concat_zeros]
        jax.block_until_ready(args)
        t0 = _time.perf_counter()
        outs = sharded(*args)
        jax.block_until_ready(outs)
        times.append(_time.perf_counter() - t0)
    return times


def kernel(x, rotary_emb, w_qkv, w_out):
    in_maps = host_prep(x, rotary_emb, w_qkv, w_out)
    res = run_on_hw(in_maps)
    y = np.empty((B, N, H), dtype=np.float32)
    for b in range(B):
        y[b] = res.results[2 * b]["y"] + res.results[2 * b + 1]["y"]
    return y

